# revision 1
# baseline (speedup 1.0000x reference)
"""Trainium2 Bass kernel for the Performer-style random-feature map:

    out[n, s] = exp(-||x_n||^2 / 2) * S^{-1/2} * exp((x @ W.T)[n, s] + b[s])
              = exp((x @ W.T)[n, s] - 0.5*||x_n||^2 - 0.5*ln(S)) * exp(b[s])

Sharding: data-parallel over the N (row) axis across 8 NeuronCores; W and b
replicated.  Each core computes a [2048, 2048] output block.  Pure SPMD, no
collectives.

Per-core structure (sizes hardcoded for N=16384, D=1024, S=2048):
  - x^T and W^T live in SBUF as bf16 k-strips of [128, *] (one tile per
    strip so matmuls only wait on the strip they need); the matmul
    contracts over d on partitions.
  - natural-layout x rows stream in per 128-row block; DVE computes
    bias_n = -0.5*||x_n||^2 - 0.5*ln(S) as a per-partition scalar.
  - per [128, 1024] PSUM group: 16 accumulating matmuls -> ACT exp(psum +
    bias_n) -> GpSimd multiply by exp(b) broadcast -> DMA out.
"""

import sys
from contextlib import ExitStack

if "/opt/trn_rl_repo" not in sys.path:
    sys.path.insert(0, "/opt/trn_rl_repo")

import numpy as np

import concourse.bacc as bacc
import concourse.bass as bass
import concourse.tile as tile
from concourse import mybir

P = 128          # SBUF partitions
N_FULL = 16384   # total rows
D_FULL = 1024    # contraction dim
S_FULL = 2048    # output features
N_CORES = 8
NC_FULL = N_FULL // N_CORES  # rows per core

F32 = mybir.dt.float32
BF16 = mybir.dt.bfloat16


def build_nc(NCc=NC_FULL, D=D_FULL, S=S_FULL, psum_w=1024,
             mm_n=512, psum_bufs=4, eb_engine="gpsimd", warmup=36,
             xn_early=3):
    """Build the single-core Bass program (same program runs SPMD on 8 cores)."""
    nc = bacc.Bacc("TRN2", target_bir_lowering=False, debug=False)

    xT = nc.dram_tensor("xT", [D, NCc], BF16, kind="ExternalInput").ap()
    xn = nc.dram_tensor("xn", [NCc, D], F32, kind="ExternalInput").ap()
    w = nc.dram_tensor("w", [D, S], BF16, kind="ExternalInput").ap()
    bv = nc.dram_tensor("bias", [S], F32, kind="ExternalInput").ap()
    out = nc.dram_tensor("out", [NCc, S], F32, kind="ExternalOutput").ap()

    KT = D // P            # k tiles (contraction)
    NB = NCc // P          # 128-row output blocks
    NS = min(mm_n, S)      # matmul moving free dim (<= 512 for one PSUM bank)
    S2 = min(psum_w, S)    # psum tile width
    SH = S // S2           # psum tiles per row block
    neg_half_ln_s = float(-0.5 * np.log(S))

    with tile.TileContext(nc) as tc, ExitStack() as ctx:
        singles = ctx.enter_context(tc.tile_pool(name="singles", bufs=1))
        w_sb = singles.tile([P, KT, S], BF16)
        x_sb = singles.tile([P, KT, NCc], BF16)
        b_bc = singles.tile([P, S], F32)
        eb = singles.tile([P, S], F32)
        bias_tiles = [
            singles.tile([P, 1], F32, tag=f"bias{nb}", name=f"bias{nb}")
            for nb in range(NB)
        ]


        # r-path: natural-layout x blocks -> per-partition exp bias.
        # First few blocks + b go on the scalar (qAct) DMA ring so the
        # early exp/mul ops have their operands; the rest of xn queues on
        # the sync ring BEHIND the matmul strips (strips get full HBM BW).
        xn_pool = ctx.enter_context(tc.tile_pool(name="xnp", bufs=4))
        sq_pool = ctx.enter_context(tc.tile_pool(name="sqp", bufs=2))
        r_pool = ctx.enter_context(tc.tile_pool(name="rp", bufs=4))
        xn_tiles = {}

        def load_xn_early(nb, eng):
            xt = xn_pool.tile([P, D], F32, tag=f"xne{nb}", name=f"xne{nb}",
                              bufs=1)
            eng.dma_start(xt, xn[nb * P:(nb + 1) * P, :])
            xn_tiles[nb] = xt

        # scalar ring: xn0, b broadcast, all of W (one big DMA), more xn
        load_xn_early(0, nc.scalar)
        bv_bcast = bass.AP(tensor=bv.tensor, offset=bv.offset,
                           ap=[[0, P]] + list(bv.ap))
        nc.scalar.dma_start(b_bc, bv_bcast)
        nc.scalar.dma_start(
            w_sb, w.rearrange("(k p) s -> p k s", p=P))
        nc.scalar.activation(eb, b_bc, func=mybir.ActivationFunctionType.Exp)
        for nb in range(1, min(xn_early, NB)):
            load_xn_early(nb, nc.scalar)

        # sync ring: all of x (one big DMA), then output tiles
        nc.sync.dma_start(
            x_sb, xT.rearrange("(k p) n -> p k n", p=P))

        def load_xn(nb):
            xt = xn_pool.tile([P, D], F32, tag="xns", name=f"xn{nb}")
            nc.scalar.dma_start(xt, xn[nb * P:(nb + 1) * P, :])
            xn_tiles[nb] = xt

        def r_bias(nb):
            xt = xn_tiles[nb]
            sq = sq_pool.tile([P, D], F32)
            nc.vector.tensor_mul(sq, xt, xt)
            r_raw = r_pool.tile([P, 1], F32)
            nc.vector.tensor_reduce(
                r_raw, sq, axis=mybir.AxisListType.X, op=mybir.AluOpType.add)
            nc.vector.tensor_scalar(
                out=bias_tiles[nb], in0=r_raw,
                scalar1=-0.5, scalar2=neg_half_ln_s,
                op0=mybir.AluOpType.mult, op1=mybir.AluOpType.add)

        for nb in range(min(xn_early + 2, NB)):
            if nb >= xn_early:
                load_xn(nb)
            if nb < min(xn_early, NB):
                r_bias(nb)

        psum_pool = ctx.enter_context(
            tc.tile_pool(name="psum", bufs=psum_bufs, space="PSUM"))
        tmp_pool = ctx.enter_context(tc.tile_pool(name="tmp", bufs=3))
        out_pool = ctx.enter_context(tc.tile_pool(name="osb", bufs=4))

        if warmup:
            # keep the PE busy (and HAM-warm) while the operand strips
            # stream in; results are discarded
            dummy_x = singles.tile([P, P], BF16)
            dummy_w = singles.tile([P, NS], BF16)
            nc.vector.memset(dummy_x, 0.0)
            nc.vector.memset(dummy_w, 0.0)
            for i in range(warmup):
                wps = psum_pool.tile([P, S2], F32, tag="ps", name=f"warm{i}")
                nc.tensor.matmul(wps[:, 0:NS], lhsT=dummy_x, rhs=dummy_w,
                                 start=True, stop=True)

        for nb in range(NB):
            nxt = nb + xn_early + 2
            if nxt < NB:
                load_xn(nxt)
            for h in range(SH):
                ps = psum_pool.tile([P, S2], F32, tag="ps", name=f"ps{nb}_{h}")
                for c in range(S2 // NS):
                    col0 = h * S2 + c * NS
                    for k in range(KT):
                        nc.tensor.matmul(
                            ps[:, c * NS:(c + 1) * NS],
                            lhsT=x_sb[:, k, nb * P:(nb + 1) * P],
                            rhs=w_sb[:, k, col0:col0 + NS],
                            start=(k == 0),
                            stop=(k == KT - 1),
                        )
                tmp = tmp_pool.tile([P, S2], F32)
                nc.scalar.activation(
                    tmp, ps,
                    func=mybir.ActivationFunctionType.Exp,
                    bias=bias_tiles[nb],
                    scale=1.0,
                )
                hsl = slice(h * S2, (h + 1) * S2)
                o_sb = out_pool.tile([P, S2], F32)
                eng = nc.gpsimd if (eb_engine == "gpsimd" and h % 2 == 0) \
                    else nc.vector
                eng.tensor_mul(o_sb, tmp, eb[:, hsl])
                nc.sync.dma_start(out[nb * P:(nb + 1) * P, hsl], o_sb)
            if nb + 3 < NB:
                r_bias(nb + 3)

    nc.compile()
    return nc


_NC_CACHE = {}


def _get_nc(**kwargs):
    key = tuple(sorted(kwargs.items()))
    if key not in _NC_CACHE:
        _NC_CACHE[key] = build_nc(**kwargs)
    return _NC_CACHE[key]


def make_in_maps(x, W, b):
    import ml_dtypes
    bf16 = ml_dtypes.bfloat16
    wT = np.ascontiguousarray(W.T.astype(bf16))
    b = np.ascontiguousarray(b.astype(np.float32))
    in_maps = []
    for i in range(N_CORES):
        xs = np.ascontiguousarray(x[i * NC_FULL:(i + 1) * NC_FULL].astype(np.float32))
        in_maps.append({
            "xT": np.ascontiguousarray(xs.T.astype(bf16)),
            "xn": xs,
            "w": wT,
            "bias": b,
        })
    return in_maps


def run_hw(x, W, b, trace=False, **build_kwargs):
    """Run on 8 NeuronCores; returns (out [N, S] f32, BassKernelResults)."""
    from concourse.bass_utils import run_bass_kernel_spmd
    from concourse.bass_interp import get_hw_module

    nc = _get_nc(**build_kwargs)
    in_maps = make_in_maps(x, W, b)
    old_m = nc.m
    nc.m = get_hw_module(nc.m)
    try:
        res = run_bass_kernel_spmd(
            nc, in_maps, core_ids=list(range(N_CORES)), trace=trace)
    finally:
        nc.m = old_m
    out = np.concatenate(
        [res.results[i]["out"] for i in range(N_CORES)], axis=0)
    return out.astype(np.float32), res


def kernel(x, W, b):
    out, _ = run_hw(x, W, b, trace=False)
    return out



# revision 5
# speedup vs baseline: 1.7847x; 1.7847x over previous
"""Trainium2 Bass kernel for the Performer-style random-feature map:

    out[n, s] = exp(-||x_n||^2 / 2) * S^{-1/2} * exp((x @ W.T)[n, s] + b[s])
              = exp((x @ W.T)[n, s] - 0.5*||x_n||^2 - 0.5*ln(S)) * exp(b[s])

Sharding: data-parallel over the N (row) axis across 8 NeuronCores; W and b
replicated.  Each core computes a [2048, 2048] output block.  Pure SPMD, no
collectives.

v2 (fp8 DoubleRow):
  - matmul in fp8e4 with perf_mode=DoubleRow: 256-deep contraction per
    instruction, 2 MACs/cell/cycle -> ~1.5x the bf16 PE roofline.  W is
    pre-scaled by 32 on the host so its values sit in e4m3's normal range;
    the 1/32 is folded into the ACT exp scale.
  - per [128, 2048] row block: 16 DoubleRow matmuls fill 4 PSUM banks,
    one ACT exp(psum/32 + bias_n) -> bf16, one DVE multiply by exp(b)
    broadcast, DMA out.  PSUM ping-pongs 2 x 4 banks.
  - x ships once as fp8 k-strips [128, 8, 2048] for the matmul plus once
    as bf16 natural rows for the row-norm (DVE tensor_tensor_reduce fuses
    square+reduce+affine into bias_n in one instruction).
  - output is bf16 on device (<=0.4% quantization, far under the 2e-2
    gate), widened to f32 on the host during the gather.
  - input DMAs are chunked by k-pair so the first matmuls only wait on
    the first 1MB; dummy fp8 matmuls warm the PE clock (HAM) while the
    operand strips stream in.
"""

import sys
from contextlib import ExitStack

if "/opt/trn_rl_repo" not in sys.path:
    sys.path.insert(0, "/opt/trn_rl_repo")

import numpy as np

import concourse.bacc as bacc
import concourse.bass as bass
import concourse.tile as tile
from concourse import mybir

P = 128          # SBUF partitions
N_FULL = 16384   # total rows
D_FULL = 1024    # contraction dim
S_FULL = 2048    # output features
N_CORES = 8
NC_FULL = N_FULL // N_CORES  # rows per core
W_SCALE = 32.0   # host pre-scale on W so fp8 e4m3 sees ~N(0,1) values

F32 = mybir.dt.float32
BF16 = mybir.dt.bfloat16
F8 = mybir.dt.float8e4
DR = mybir.MatmulPerfMode.DoubleRow


def build_nc(NCc=NC_FULL, D=D_FULL, S=S_FULL, warmup=24, xn_ahead=3,
             bias_ahead=2):
    """Build the single-core Bass program (same program runs SPMD on 8 cores)."""
    nc = bacc.Bacc("TRN2", target_bir_lowering=False, debug=False)

    xT = nc.dram_tensor("xT8", [D, NCc], F8, kind="ExternalInput").ap()
    xn = nc.dram_tensor("xn", [NCc, D], BF16, kind="ExternalInput").ap()
    w = nc.dram_tensor("w8", [D, S], F8, kind="ExternalInput").ap()
    bv = nc.dram_tensor("bias", [S], F32, kind="ExternalInput").ap()
    out = nc.dram_tensor("out", [NCc, S], BF16, kind="ExternalOutput").ap()

    KT = D // P            # 8 k strips of 128
    K2 = KT // 2           # 4 DoubleRow chunks of 256
    NB = NCc // P          # 128-row output blocks
    NS = 512               # matmul moving free dim (one PSUM bank fp32)
    SH = S // NS           # s-chunks per row block
    neg_half_ln_s = float(-0.5 * np.log(S))

    with tile.TileContext(nc) as tc, ExitStack() as ctx:
        singles = ctx.enter_context(tc.tile_pool(name="singles", bufs=1))
        w_sb = singles.tile([P, KT, S], F8)
        x_sb = singles.tile([P, KT, NCc], F8)
        b_bc = singles.tile([P, S], F32)
        eb = singles.tile([P, S], BF16)
        bias_tiles = [
            singles.tile([P, 1], F32, tag=f"bias{nb}", name=f"bias{nb}")
            for nb in range(NB)
        ]

        # warm-up dummies (no DMA dependency -> PE starts immediately)
        dx = singles.tile([P, 2, P], F8)
        dw = singles.tile([P, 2, NS], F8)
        nc.vector.memset(dx, 0.0)
        nc.vector.memset(dw, 0.0)

        xn_pool = ctx.enter_context(tc.tile_pool(name="xnp", bufs=4))
        sq_pool = ctx.enter_context(tc.tile_pool(name="sqp", bufs=2))
        r_pool = ctx.enter_context(tc.tile_pool(name="rp", bufs=4))
        psum_pool = ctx.enter_context(
            tc.tile_pool(name="psum", bufs=2, space="PSUM"))
        tmp_pool = ctx.enter_context(tc.tile_pool(name="tmp", bufs=2))
        out_pool = ctx.enter_context(tc.tile_pool(name="osb", bufs=3))

        # ring B (scalar queue): W k-pair chunks, then b broadcast
        wr = w.rearrange("(k p) s -> p k s", p=P)
        for k2 in range(K2):
            nc.scalar.dma_start(w_sb[:, 2 * k2:2 * k2 + 2, :],
                                wr[:, 2 * k2:2 * k2 + 2, :])
        bv_bcast = bass.AP(tensor=bv.tensor, offset=bv.offset,
                           ap=[[0, P]] + list(bv.ap))
        nc.scalar.dma_start(b_bc, bv_bcast)
        nc.scalar.activation(eb, b_bc, func=mybir.ActivationFunctionType.Exp)

        # ring A (sync queue): x k-pair chunks, then xn rows + output tiles
        xr = xT.rearrange("(k p) n -> p k n", p=P)
        for k2 in range(K2):
            nc.sync.dma_start(x_sb[:, 2 * k2:2 * k2 + 2, :],
                              xr[:, 2 * k2:2 * k2 + 2, :])

        xn_tiles = {}

        def load_xn(nb):
            xt = xn_pool.tile([P, D], BF16, tag="xns", name=f"xn{nb}")
            nc.sync.dma_start(xt, xn[nb * P:(nb + 1) * P, :])
            xn_tiles[nb] = xt

        def r_bias(nb):
            # bias_n = -0.5*||x_n||^2 - 0.5*ln(S)
            # (tensor_tensor_reduce would fuse this but dies on HW with an
            # INTERNAL error; use the proven 3-op DVE sequence instead)
            xt = xn_tiles.pop(nb)
            sq = sq_pool.tile([P, D], BF16)
            nc.vector.tensor_mul(sq, xt, xt)
            r_raw = r_pool.tile([P, 1], F32)
            nc.vector.tensor_reduce(
                r_raw, sq, axis=mybir.AxisListType.X, op=mybir.AluOpType.add)
            nc.vector.tensor_scalar(
                out=bias_tiles[nb], in0=r_raw,
                scalar1=-0.5, scalar2=neg_half_ln_s,
                op0=mybir.AluOpType.mult, op1=mybir.AluOpType.add)

        for nb in range(min(xn_ahead, NB)):
            load_xn(nb)
        for nb in range(min(bias_ahead, NB)):
            r_bias(nb)

        # keep the PE busy (and HAM-warm) while the operand strips stream in
        for i in range(warmup):
            wps = psum_pool.tile([P, S], F32, tag="ps", name=f"warm{i}")
            nc.tensor.matmul(wps[:, 0:NS], lhsT=dx, rhs=dw,
                             start=True, stop=True, perf_mode=DR)

        for nb in range(NB):
            if nb + xn_ahead < NB:
                load_xn(nb + xn_ahead)
            ps = psum_pool.tile([P, S], F32, tag="ps", name=f"ps{nb}")
            for k2 in range(K2):
                lt = x_sb[:, 2 * k2:2 * k2 + 2, nb * P:(nb + 1) * P]
                for h in range(SH):
                    nc.tensor.matmul(
                        ps[:, h * NS:(h + 1) * NS],
                        lhsT=lt,
                        rhs=w_sb[:, 2 * k2:2 * k2 + 2, h * NS:(h + 1) * NS],
                        start=(k2 == 0),
                        stop=(k2 == K2 - 1),
                        perf_mode=DR)
            tmp = tmp_pool.tile([P, S], BF16)
            nc.scalar.activation(
                tmp, ps,
                func=mybir.ActivationFunctionType.Exp,
                bias=bias_tiles[nb],
                scale=1.0 / W_SCALE)
            o_sb = out_pool.tile([P, S], BF16)
            nc.vector.tensor_mul(o_sb, tmp, eb)
            nc.sync.dma_start(out[nb * P:(nb + 1) * P, :], o_sb)
            if nb + bias_ahead < NB:
                r_bias(nb + bias_ahead)

    nc.compile()
    return nc


_NC_CACHE = {}


def _get_nc(**kwargs):
    key = tuple(sorted(kwargs.items()))
    if key not in _NC_CACHE:
        _NC_CACHE[key] = build_nc(**kwargs)
    return _NC_CACHE[key]


def make_in_maps(x, W, b):
    import ml_dtypes
    bf16 = ml_dtypes.bfloat16
    f8 = ml_dtypes.float8_e4m3
    w8 = np.ascontiguousarray(
        (W.T.astype(np.float32) * W_SCALE).astype(f8))
    bf = np.ascontiguousarray(b.astype(np.float32))
    in_maps = []
    for i in range(N_CORES):
        xs = np.ascontiguousarray(
            x[i * NC_FULL:(i + 1) * NC_FULL].astype(np.float32))
        in_maps.append({
            "xT8": np.ascontiguousarray(xs.T.astype(f8)),
            "xn": np.ascontiguousarray(xs.astype(bf16)),
            "w8": w8,
            "bias": bf,
        })
    return in_maps


def run_hw(x, W, b, trace=False, **build_kwargs):
    """Run on 8 NeuronCores; returns (out [N, S] f32, BassKernelResults)."""
    from concourse.bass_utils import run_bass_kernel_spmd
    from concourse.bass_interp import get_hw_module

    nc = _get_nc(**build_kwargs)
    in_maps = make_in_maps(x, W, b)
    old_m = nc.m
    nc.m = get_hw_module(nc.m)
    try:
        res = run_bass_kernel_spmd(
            nc, in_maps, core_ids=list(range(N_CORES)), trace=trace)
    finally:
        nc.m = old_m
    out = np.concatenate(
        [res.results[i]["out"].astype(np.float32) for i in range(N_CORES)],
        axis=0)
    return out, res


def kernel(x, W, b):
    out, _ = run_hw(x, W, b, trace=False)
    return out
